# revision 1
# baseline (speedup 1.0000x reference)
"""Cost-sensitive cross-entropy loss on 8 TRN2 NeuronCores (Bass/Tile).

Data-parallel over the batch: each core streams its 8192x1000 logit shard;
per [128,1000] tile it computes row max (reduce_max), argmax via an
equality-mask * reverse-iota reduce_max, the softmax denominator via the
scalar engine's Exp with fused accumulation, and the target logit via a
per-tile indirect-DMA gather (one offset per partition).  Each sample then
contributes an ENC-encoded one-hot payload (count and value packed in one
f32) which is scatter-added into per-core HBM bins keyed by
(target*16 + predicted>>6).  dma_scatter_add does not serialize
read-modify-writes of the same 256B row within a call, so same-row
duplicates inside each 128-record tile are merged on the tensor engine
(equality-matrix matmul), non-first duplicates are routed to a garbage
row, and the per-tile calls form two WAW-serialized chains into separate
buffers that are then locally summed.  A ReduceScatter combines bins
across cores (class-sharded); each core decodes counts/values,
row-normalizes against the cost matrix and emits a partial loss-sum; the
host sums 8 scalars and applies -beta/B.

Self-contained: hardcodes B=65536, C=1000, beta=3.0, 8 cores.
"""
import sys
sys.path.insert(0, '/opt/trn_rl_repo')
import numpy as np
import concourse.bass as bass
import concourse.bacc as bacc
import concourse.mybir as mybir
from concourse import tile
from concourse.masks import make_identity
from concourse.bass_utils import run_bass_kernel_spmd

P = 128
C = 1000
B = 65536
N_CORES = 8
R = B // N_CORES
SUB = 16          # 64-wide sub-rows per class row (16*64 = 1024 >= C)
CW = 64           # scatter row width (f32) = 256B
ENC = 4096.0      # count encoding scale
SINGLE_PACKET = False
BETA = 3.0

F32 = mybir.dt.float32
I32 = mybir.dt.int32
I16 = mybir.dt.int16
U32 = mybir.dt.uint32
AF = mybir.ActivationFunctionType
ALU = mybir.AluOpType
AX = mybir.AxisListType


def build_kernel(R, n_cores, my_rank=None, dma_bufs=4, debug=False):
    NT = R // P
    NROW = C * SUB
    SR = NROW // n_cores
    Tsh = C // n_cores

    nc = bacc.Bacc(None, target_bir_lowering=False)
    xs = nc.dram_tensor("xs", [R, C], F32, kind="ExternalInput")
    tg = nc.dram_tensor("tg", [R], I32, kind="ExternalInput")
    cmrow = nc.dram_tensor("cmrow", [Tsh, C], F32, kind="ExternalInput")
    out = nc.dram_tensor("partial", [1, 1], F32, kind="ExternalOutput")

    bins_a = nc.dram_tensor("bins_a", [NROW + 1, CW], F32)
    bins_b = nc.dram_tensor("bins_b", [NROW + 1, CW], F32)
    rs_out = nc.dram_tensor("rs_out", [SR, CW], F32)
    red = nc.dram_tensor("red", [1, 2 * P], F32)
    if debug:
        dbg_m = nc.dram_tensor("dbg_m", [P, NT], F32, kind="ExternalOutput")
        dbg_p = nc.dram_tensor("dbg_p", [P, NT], U32, kind="ExternalOutput")
        dbg_s = nc.dram_tensor("dbg_s", [P, NT], F32, kind="ExternalOutput")
        dbg_xt = nc.dram_tensor("dbg_xt", [P, NT], F32, kind="ExternalOutput")
        dbg_bins = nc.dram_tensor("dbg_bins", [NROW, CW], F32, kind="ExternalOutput")

    xs_flat = xs[:].rearrange("a b -> (a b)")[:, None]
    za = bins_a[0:NROW, :].rearrange("(p n) c -> p (n c)", p=P)
    zb = bins_b[0:NROW, :].rearrange("(p n) c -> p (n c)", p=P)
    ZW = za.shape[1]
    H = ZW // 2

    with tile.TileContext(nc) as tc:
        with (
            tc.tile_pool(name="xp", bufs=dma_bufs) as xp,
            tc.tile_pool(name="sp", bufs=2) as sp,
            tc.tile_pool(name="pp", bufs=1) as pp,
            tc.tile_pool(name="ps", bufs=2, space="PSUM") as psp,
        ):
            # --- constants ---
            zt = pp.tile([P, H], F32)
            nc.vector.memset(zt[:], 0.0)
            nc.sync.dma_start(out=za[:, :H], in_=zt[:])
            nc.sync.dma_start(out=za[:, H:], in_=zt[:, : ZW - H])
            nc.sync.dma_start(out=zb[:, :H], in_=zt[:])
            nc.sync.dma_start(out=zb[:, H:], in_=zt[:, : ZW - H])

            iota64_i = pp.tile([P, CW], I32)
            nc.gpsimd.iota(iota64_i[:], pattern=[[1, CW]], base=0, channel_multiplier=0)
            iota64 = pp.tile([P, CW], F32)
            nc.vector.tensor_copy(iota64[:], iota64_i[:])

            ident = pp.tile([P, P], F32)
            make_identity(nc, ident[:])
            # strict-lower-triangle mask for first-occurrence detection
            iov_i = pp.tile([P, 1], I32)
            nc.gpsimd.iota(iov_i[:], pattern=[[1, 1]], base=0, channel_multiplier=1)
            iov = pp.tile([P, 1], F32)
            nc.vector.tensor_copy(iov[:], iov_i[:])
            jT_ps = psp.tile([P, P], F32)
            nc.tensor.transpose(jT_ps[:], iov[:].to_broadcast([P, P]), identity=ident[:])
            ltri = pp.tile([P, P], F32)
            nc.vector.tensor_tensor(
                ltri[:], iov[:].to_broadcast([P, P]), jT_ps[:], op=ALU.is_gt
            )

            m8_all = pp.tile([P, NT * 8], F32)
            p8_all = pp.tile([P, NT * 8], U32)
            s_all = pp.tile([P, NT], F32)

            # targets + gather offsets
            t_sb = pp.tile([P, NT], I32)
            nc.sync.dma_start(out=t_sb[:], in_=tg[:].rearrange("(n p) -> p n", p=P))
            io_r = pp.tile([P, NT], I32)
            nc.gpsimd.iota(io_r[:], pattern=[[1, NT]], base=0, channel_multiplier=0)
            io_pc = pp.tile([P, 1], I32)
            nc.gpsimd.iota(io_pc[:], pattern=[[1, 1]], base=0, channel_multiplier=C)
            off = pp.tile([P, NT], I32)
            nc.vector.tensor_scalar(off[:], io_r[:], P * C, None, op0=ALU.mult)
            nc.vector.tensor_tensor(off[:], off[:], t_sb[:], op=ALU.add)
            nc.vector.tensor_tensor(
                off[:], off[:], io_pc[:].to_broadcast([P, NT]), op=ALU.add
            )
            x_t = pp.tile([P, NT], F32)

            # --- phase 1 main loop ---
            for r in range(NT):
                x = xp.tile([P, C], F32)
                nc.sync.dma_start(out=x[:], in_=xs[:].rearrange("(n p) c -> n p c", p=P)[r])
                nc.vector.max(m8_all[:, 8 * r : 8 * r + 8], x[:])
                nc.vector.max_index(
                    p8_all[:, 8 * r : 8 * r + 8], m8_all[:, 8 * r : 8 * r + 8], x[:]
                )
                e = xp.tile([P, C], F32, tag="e")
                nc.scalar.activation(
                    e[:], x[:], AF.Exp, accum_out=s_all[:, r : r + 1]
                )
                nc.gpsimd.indirect_dma_start(
                    out=x_t[:, r : r + 1], out_offset=None,
                    in_=xs_flat,
                    in_offset=bass.IndirectOffsetOnAxis(ap=off[:, r : r + 1], axis=0),
                )

            # --- per-sample post pass ---
            logs = pp.tile([P, NT], F32)
            nc.scalar.activation(logs[:], s_all[:], AF.Ln)
            venc = pp.tile([P, NT], F32)
            nc.vector.tensor_tensor(venc[:], x_t[:], logs[:], op=ALU.subtract)
            nc.vector.tensor_scalar_add(venc[:], venc[:], ENC)

            # k' = t*16 + (p>>6); c6 = p & 63
            pu = pp.tile([P, NT], U32)
            nc.vector.tensor_copy(
                pu[:], p8_all[:].rearrange("p (r e) -> p r e", e=8)[:, :, 0]
            )
            hi = pp.tile([P, NT], U32)
            nc.vector.tensor_scalar(hi[:], pu[:], 6, None, op0=ALU.logical_shift_right)
            hi_i = pp.tile([P, NT], I32)
            nc.vector.tensor_copy(hi_i[:], hi[:])
            c6 = pp.tile([P, NT], U32)
            nc.vector.tensor_scalar(c6[:], pu[:], 63, None, op0=ALU.bitwise_and)
            c6f = pp.tile([P, NT], F32)
            nc.vector.tensor_copy(c6f[:], c6[:])
            kp = pp.tile([P, NT], I32)
            nc.vector.tensor_scalar(kp[:], t_sb[:], SUB, None, op0=ALU.mult)
            nc.vector.tensor_tensor(kp[:], kp[:], hi_i[:], op=ALU.add)
            kpf = pp.tile([P, NT], F32)
            nc.vector.tensor_copy(kpf[:], kp[:])
            kpm = pp.tile([P, NT], F32)
            nc.vector.tensor_scalar_add(kpm[:], kpf[:], float(-NROW))

            # payload [P, NT, 64] = (iota64 == c6) * venc
            payload = pp.tile([P, NT * CW], F32)
            pl3 = payload[:].rearrange("p (r c) -> p r c", c=CW)
            nc.vector.tensor_tensor(
                pl3, iota64[:, None, :].to_broadcast([P, NT, CW]),
                c6f[:].to_broadcast([P, NT, CW]), op=ALU.is_equal,
            )
            nc.vector.tensor_tensor(
                pl3, pl3, venc[:].to_broadcast([P, NT, CW]), op=ALU.mult,
            )

            # --- per-tile duplicate merge, two pipelined halves ---------
            # half h: dedup tiles -> convert idx -> shuffle -> scatter chain;
            # half 1's scatters overlap half 2's dedup.
            kadj = pp.tile([P, NT], F32)
            pay2 = pp.tile([P, NT * CW], F32)
            p23 = pay2[:].rearrange("p (r c) -> p r c", c=CW)
            HT = max(1, NT // 8)
            for half in range(0, NT, HT):
                for r in range(half, half + HT):
                    kT = psp.tile([P, P], F32, tag="kT")
                    nc.tensor.transpose(
                        kT[:], kpf[:, r : r + 1].to_broadcast([P, P]), identity=ident[:]
                    )
                    eq = sp.tile([P, P], F32, tag="eq")
                    nc.vector.tensor_tensor(
                        eq[:], kpf[:, r : r + 1].to_broadcast([P, P]), kT[:],
                        op=ALU.is_equal,
                    )
                    mg = psp.tile([P, CW], F32, tag="mg")
                    nc.tensor.matmul(mg[:], lhsT=eq[:], rhs=pl3[:, r, :], start=True, stop=True)
                    eql = sp.tile([P, P], F32, tag="eql")
                    nc.vector.tensor_tensor(eql[:], eq[:], ltri[:], op=ALU.mult)
                    prev = sp.tile([P, 1], F32, tag="prev")
                    nc.vector.reduce_max(prev[:], eql[:], axis=AX.X)
                    keep = sp.tile([P, 1], F32, tag="keep")
                    nc.vector.tensor_scalar(keep[:], prev[:], 0.0, None, op0=ALU.is_equal)
                    nc.vector.tensor_scalar(
                        kadj[:, r : r + 1], kpm[:, r : r + 1], keep[:], float(NROW),
                        op0=ALU.mult, op1=ALU.add,
                    )
                    nc.scalar.copy(p23[:, r, :], mg[:])

                kp16 = sp.tile([P, HT], I16, tag="kp16")
                nc.vector.tensor_copy(kp16[:], kadj[:, half : half + HT])
                idx = sp.tile([P, HT * 8], I16, tag="idx")
                idx_v = idx[:].rearrange("p (r e) -> p r e", e=8)
                for h in range(8):
                    nc.sync.dma_start(
                        out=idx_v[0:16, :, h], in_=kp16[16 * h : 16 * h + 16, :]
                    )
                for g in range(1, 8):
                    nc.sync.dma_start(out=idx[16 * g : 16 * (g + 1), :], in_=idx[0:16, :])

                for r in range(half, half + HT):
                    dst = bins_a if r % 2 == 0 else bins_b
                    rr = r - half
                    nc.gpsimd.dma_scatter_add(
                        dst[:], p23[:, r : r + 1, :], idx[:, rr * 8 : (rr + 1) * 8],
                        num_idxs=P, num_idxs_reg=P, elem_size=CW,
                        single_packet=SINGLE_PACKET,
                    )

            # merge b into a
            for h in range(2):
                sl = slice(h * H, h * H + H)
                ma = pp.tile([P, H], F32, tag="ma")
                mb = pp.tile([P, H], F32, tag="mb")
                nc.sync.dma_start(out=ma[:], in_=za[:, sl])
                nc.sync.dma_start(out=mb[:], in_=zb[:, sl])
                nc.vector.tensor_tensor(ma[:], ma[:], mb[:], op=ALU.add)
                nc.sync.dma_start(out=za[:, sl], in_=ma[:])

            if debug:
                nc.sync.dma_start(out=dbg_m[:], in_=m8_all[:].rearrange("p (r e) -> p r e", e=8)[:, :, 0])
                nc.sync.dma_start(out=dbg_p[:], in_=pu[:])
                nc.sync.dma_start(out=dbg_s[:], in_=s_all[:])
                nc.sync.dma_start(out=dbg_xt[:], in_=x_t[:])
                nc.sync.dma_start(out=dbg_bins[:], in_=bins_a[0:NROW, :])

            # --- reduce across cores ---
            if n_cores > 1:
                nc.gpsimd.collective_compute(
                    "ReduceScatter", ALU.add,
                    replica_groups=[list(range(n_cores))],
                    ins=[bins_a[0:NROW, :].rearrange("a b -> (a b)")],
                    outs=[rs_out[:].rearrange("a b -> (a b)")],
                )
                shard = rs_out[:]
            else:
                shard = bins_a[0:NROW, :]

            # --- phase 3 ---
            sh_v = shard.rearrange("(t s) c -> t (s c)", s=SUB)
            n_tt = (Tsh + P - 1) // P
            parts = pp.tile([P, n_tt], F32)
            nc.vector.memset(parts[:], 0.0)
            for tt in range(n_tt):
                lo = tt * P
                hi_t = min(Tsh, lo + P)
                npart = hi_t - lo
                S = pp.tile([P, SUB * CW], F32, tag="s3")
                if npart < P:
                    nc.vector.memset(S[:], 0.0)
                nc.sync.dma_start(out=S[:npart], in_=sh_v[lo:hi_t])
                cmt = pp.tile([P, SUB * CW], F32, tag="cm3")
                nc.vector.memset(cmt[:], 0.0)
                nc.sync.dma_start(out=cmt[:npart, 0:C], in_=cmrow[lo:hi_t, :])
                cnt_i = pp.tile([P, SUB * CW], I32, tag="ci3")
                sdiv = pp.tile([P, SUB * CW], F32, tag="sd3")
                nc.vector.tensor_scalar(
                    sdiv[:], S[:], 1.0 / ENC, 0.499, op0=ALU.mult, op1=ALU.add
                )
                nc.vector.tensor_copy(cnt_i[:], sdiv[:])
                cntf = pp.tile([P, SUB * CW], F32, tag="cf3")
                nc.vector.tensor_copy(cntf[:], cnt_i[:])
                encc = pp.tile([P, SUB * CW], F32, tag="ec3")
                nc.vector.tensor_scalar_mul(encc[:], cntf[:], ENC)
                nc.vector.tensor_tensor(S[:], S[:], encc[:], op=ALU.subtract)
                nc.vector.tensor_tensor(cmt[:], cmt[:], cntf[:], op=ALU.add)
                rows = pp.tile([P, 1], F32, tag="rs3")
                nc.vector.reduce_sum(rows[:], cmt[:], axis=AX.X)
                nc.vector.tensor_scalar_max(rows[:], rows[:], 1.0)
                rec = pp.tile([P, 1], F32, tag="rc3")
                nc.vector.reciprocal(rec[:], rows[:])
                nc.vector.tensor_tensor(cmt[:], cmt[:], S[:], op=ALU.mult)
                ws = pp.tile([P, 1], F32, tag="ws3")
                nc.vector.reduce_sum(ws[:], cmt[:], axis=AX.X)
                nc.vector.tensor_tensor(parts[:, tt : tt + 1], ws[:], rec[:], op=ALU.mult)

            colsum = pp.tile([P, 1], F32)
            nc.vector.reduce_sum(colsum[:], parts[:], axis=AX.X)
            nc.sync.dma_start(out=red[0, 0:P], in_=colsum[:, 0])
            rowt = pp.tile([1, P], F32)
            nc.sync.dma_start(out=rowt[:], in_=red[0:1, 0:P])
            nc.vector.reduce_sum(colsum[0:1, 0:1], rowt[:], axis=AX.X)
            nc.sync.dma_start(out=out[:], in_=colsum[0:1, 0:1])

    nc.finalize()
    return nc

_NC_CACHE = None


def kernel(outputs: np.ndarray, targets: np.ndarray, cost_matrix: np.ndarray) -> np.ndarray:
    global _NC_CACHE
    assert outputs.shape == (B, C) and cost_matrix.shape == (C, C)
    if _NC_CACHE is None:
        _NC_CACHE = build_kernel(R, N_CORES)
    nc = _NC_CACHE

    tg32 = np.ascontiguousarray(targets.astype(np.int32))
    xs = np.ascontiguousarray(outputs, dtype=np.float32)
    cmf = np.ascontiguousarray(cost_matrix, dtype=np.float32)
    Tsh = C // N_CORES
    in_maps = [
        {
            "xs": xs[c * R : (c + 1) * R],
            "tg": tg32[c * R : (c + 1) * R],
            "cmrow": cmf[c * Tsh : (c + 1) * Tsh],
        }
        for c in range(N_CORES)
    ]
    res = run_bass_kernel_spmd(nc, in_maps, core_ids=list(range(N_CORES)))
    total = sum(float(res.results[c]["partial"][0, 0]) for c in range(N_CORES))
    loss = -BETA * total / B
    return np.asarray(loss, dtype=np.float32)



# revision 7
# speedup vs baseline: 1.9395x; 1.9395x over previous
"""Cost-sensitive cross-entropy loss on 8 TRN2 NeuronCores (Bass/Tile).

Target-sharded data parallel: the host sorts samples by target class and
partitions the 1000 classes into 8 contiguous groups (~8192 samples each),
so every (target, predicted) collision pair lives on exactly one core --
no collective is needed; the host sums 8 partial scalars.

Host-side precompute (targets and cost_matrix are inputs, so this is
legal and exact): the per-class histogram h[t], row normalizers
1/max(1, cost_row_sum[t] + h[t]), the target logit gather x[b, t_b], and
per-sample scatter keys t*1024.  The device computes, per [128, 1000]
fp16 logit tile: argmax (vector max/max_index), sum(exp(x)) (scalar Exp
with fused accumulation), the pair-collision count n_b (key equality
against a +-1-tile window -- sorted targets guarantee same-target samples
are within 128 positions), and an indirect-DMA gather of
cost_matrix[t_b, p_b].  Loss partial = sum v_b*(n_b + cost_b)*w_b.

Logits are uploaded as fp16 (halves the HBM stream; argmax flips only
where the top-2 gap < 1 fp16 ulp, and lse error is ~2e-3 absolute -- both
far inside the 2e-2 tolerance).

Self-contained: hardcodes B=65536, C=1000, beta=3.0, 8 cores.
"""
import sys
sys.path.insert(0, '/opt/trn_rl_repo')
import numpy as np
import concourse.bass as bass
import concourse.bacc as bacc
import concourse.mybir as mybir
from concourse import tile
from concourse.masks import make_identity
from concourse.bass_utils import run_bass_kernel_spmd

P = 128
C = 1000
B = 65536
N_CORES = 8
CAP = 8448            # per-core sample capacity (66 tiles of 128)
NT = CAP // P         # 66
LMAX = 192            # max classes per core (padded)
PADKEY = float(2 << 20)
BETA = 3.0

F32 = mybir.dt.float32
F16 = mybir.dt.float16
I32 = mybir.dt.int32
U32 = mybir.dt.uint32
AF = mybir.ActivationFunctionType
ALU = mybir.AluOpType
AX = mybir.AxisListType

GCH = 1               # gather chunk: tiles per indirect-DMA call


def build_kernel(debug=False, NT=NT):
    CAP = NT * P
    nc = bacc.Bacc(None, target_bir_lowering=False)
    xh = nc.dram_tensor("xh", [CAP, C], F16, kind="ExternalInput")
    xt = nc.dram_tensor("xt", [P, NT], F32, kind="ExternalInput")
    keyt = nc.dram_tensor("keyt", [P, NT], F32, kind="ExternalInput")
    wv = nc.dram_tensor("wv", [P, NT], F32, kind="ExternalInput")
    cmbase = nc.dram_tensor("cmbase", [P, NT], I32, kind="ExternalInput")
    cmr = nc.dram_tensor("cmr", [LMAX, C], F32, kind="ExternalInput")
    out = nc.dram_tensor("partial", [1, 1], F32, kind="ExternalOutput")
    red = nc.dram_tensor("red", [1, P], F32)
    if debug:
        dbg_p = nc.dram_tensor("dbg_p", [P, NT], F32, kind="ExternalOutput")
        dbg_s = nc.dram_tensor("dbg_s", [P, NT], F32, kind="ExternalOutput")
        dbg_n = nc.dram_tensor("dbg_n", [P, NT], F32, kind="ExternalOutput")
        dbg_v = nc.dram_tensor("dbg_v", [P, NT], F32, kind="ExternalOutput")

    cm_flat = cmr[:].rearrange("a b -> (a b)")[:, None]
    xh_t = xh[:].rearrange("(n p) c -> n p c", p=P)

    with tile.TileContext(nc) as tc:
        with (
            tc.tile_pool(name="xp", bufs=4) as xp,
            tc.tile_pool(name="ep", bufs=2) as ep,
            tc.tile_pool(name="sp", bufs=3) as sp,
            tc.tile_pool(name="pp", bufs=1) as pp,
            tc.tile_pool(name="ps", bufs=3, space="PSUM") as psp,
        ):
            ident = pp.tile([P, P], F32)
            make_identity(nc, ident[:])

            # persistent [P, NT] state
            xt_sb = pp.tile([P, NT], F32)
            keyt_sb = pp.tile([P, NT], F32)
            wv_sb = pp.tile([P, NT], F32)
            cmb_sb = pp.tile([P, NT], I32)
            nc.sync.dma_start(out=xt_sb[:], in_=xt[:])
            nc.sync.dma_start(out=keyt_sb[:], in_=keyt[:])
            nc.sync.dma_start(out=wv_sb[:], in_=wv[:])
            nc.sync.dma_start(out=cmb_sb[:], in_=cmbase[:])

            s_all = pp.tile([P, NT], F32)
            key_all = pp.tile([P, NT], F32)
            cmoff = pp.tile([P, NT], I32)
            ncnt = pp.tile([P, NT], F32)
            cg = pp.tile([P, NT], F32)

            # broadcast keys ring: block j+1 holds tile j's keys; ends = -1
            kT_all = pp.tile([P, (NT + 2) * P], F32)
            nc.vector.memset(kT_all[:, 0:P], -1.0)
            nc.vector.memset(kT_all[:, (NT + 1) * P:], -1.0)

            def eq_count(r):
                # n = #{samples in tiles r-1..r+1 with equal key} (incl. self)
                eqs = sp.tile([P, 3 * P], F32, tag="eqs")
                nc.vector.tensor_tensor(
                    eqs[:],
                    kT_all[:, r * P: (r + 3) * P],
                    key_all[:, r: r + 1].to_broadcast([P, 3 * P]),
                    op=ALU.is_equal,
                )
                nc.vector.reduce_sum(ncnt[:, r: r + 1], eqs[:], axis=AX.X)

            for r in range(NT):
                x = xp.tile([P, C], F16)
                nc.sync.dma_start(out=x[:], in_=xh_t[r])
                m8 = sp.tile([P, 8], F16, tag="m8")
                p8 = sp.tile([P, 8], U32, tag="p8")
                nc.vector.max(m8[:], x[:])
                nc.vector.max_index(p8[:], m8[:], x[:])
                e = ep.tile([P, C], F16, tag="e")
                nc.scalar.activation(
                    e[:], x[:], AF.Exp, accum_out=s_all[:, r: r + 1]
                )
                # key = keyt + p ; cmoff = cmbase + p
                pf = sp.tile([P, 1], F32, tag="pf")
                nc.gpsimd.tensor_copy(pf[:], p8[:, 0:1])
                pi = sp.tile([P, 1], I32, tag="pi")
                nc.gpsimd.tensor_copy(pi[:], p8[:, 0:1])
                nc.gpsimd.tensor_tensor(
                    key_all[:, r: r + 1], pf[:], keyt_sb[:, r: r + 1], op=ALU.add
                )
                nc.gpsimd.tensor_tensor(
                    cmoff[:, r: r + 1], pi[:], cmb_sb[:, r: r + 1], op=ALU.add
                )
                # broadcast keys to all partitions via PE transpose
                kps = psp.tile([P, P], F32, tag="kps")
                nc.tensor.transpose(
                    kps[:], key_all[:, r: r + 1].to_broadcast([P, P]),
                    identity=ident[:],
                )
                nc.scalar.copy(kT_all[:, (r + 1) * P: (r + 2) * P], kps[:])
                # chunked cost-matrix gather
                if r % GCH == GCH - 1:
                    lo = r - (GCH - 1)
                    nc.gpsimd.indirect_dma_start(
                        out=cg[:, lo: r + 1], out_offset=None,
                        in_=cm_flat,
                        in_offset=bass.IndirectOffsetOnAxis(
                            ap=cmoff[:, lo: r + 1], axis=0
                        ),
                    )
                if r >= 1:
                    eq_count(r - 1)
            eq_count(NT - 1)

            # v = xt - ln(s);  contrib = v * (ncnt + cg) * wv;  partial = sum
            logs = pp.tile([P, NT], F32)
            nc.scalar.activation(logs[:], s_all[:], AF.Ln)
            v = pp.tile([P, NT], F32)
            nc.vector.tensor_tensor(v[:], xt_sb[:], logs[:], op=ALU.subtract)
            nc.vector.tensor_tensor(ncnt[:], ncnt[:], cg[:], op=ALU.add)
            contrib = pp.tile([P, NT], F32)
            nc.vector.tensor_tensor(contrib[:], v[:], ncnt[:], op=ALU.mult)
            nc.vector.tensor_tensor(contrib[:], contrib[:], wv_sb[:], op=ALU.mult)

            if debug:
                nc.sync.dma_start(out=dbg_p[:], in_=key_all[:])
                nc.sync.dma_start(out=dbg_s[:], in_=s_all[:])
                nc.sync.dma_start(out=dbg_n[:], in_=ncnt[:])
                nc.sync.dma_start(out=dbg_v[:], in_=v[:])

            colsum = pp.tile([P, 1], F32)
            nc.vector.reduce_sum(colsum[:], contrib[:], axis=AX.X)
            nc.sync.dma_start(out=red[0, 0:P], in_=colsum[:, 0])
            rowt = pp.tile([1, P], F32)
            nc.sync.dma_start(out=rowt[:], in_=red[0:1, 0:P])
            nc.vector.reduce_sum(colsum[0:1, 0:1], rowt[:], axis=AX.X)
            nc.sync.dma_start(out=out[:], in_=colsum[0:1, 0:1])

    nc.finalize()
    return nc


def make_inputs(outputs, targets, cost_matrix):
    """Host-side shard prep. Returns (in_maps, meta)."""
    t = np.asarray(targets).astype(np.int64)
    x = np.asarray(outputs, dtype=np.float32)
    cm = np.ascontiguousarray(cost_matrix, dtype=np.float32)
    counts = np.bincount(t, minlength=C)
    assert counts.max() <= P + 1, f"class run too long: {counts.max()}"
    rinv = (1.0 / np.maximum(1.0, cm.sum(axis=1) + counts)).astype(np.float32)

    # contiguous class partition: close a group once it reaches B/8 samples
    bounds = [0]
    acc = 0
    for cls in range(C):
        acc += int(counts[cls])
        if acc >= B // N_CORES and len(bounds) < N_CORES:
            bounds.append(cls + 1)
            acc = 0
    bounds.append(C)
    assert len(bounds) == N_CORES + 1

    order = np.argsort(t, kind="stable")
    t_sorted = t[order]
    x16 = x.astype(np.float16)

    def shard(c):
        c0, c1 = bounds[c], bounds[c + 1]
        assert c1 - c0 <= LMAX
        s0, s1 = np.searchsorted(t_sorted, [c0, c1])
        idx = order[s0:s1]
        n = len(idx)
        assert n <= CAP, f"core {c}: {n} > {CAP}"
        ti = t[idx]
        xh = np.zeros((CAP, C), dtype=np.float16)
        xh[:n] = x16[idx]
        xtf = np.zeros(CAP, dtype=np.float32)
        xtf[:n] = x[idx, ti]
        ktf = np.empty(CAP, dtype=np.float32)
        ktf[:n] = ti * 1024.0
        ktf[n:] = PADKEY + 1024.0 * np.arange(CAP - n, dtype=np.float32)
        wvf = np.zeros(CAP, dtype=np.float32)
        wvf[:n] = rinv[ti]
        cbf = np.zeros(CAP, dtype=np.int32)
        cbf[:n] = (ti - c0) * C
        cmr = np.zeros((LMAX, C), dtype=np.float32)
        cmr[: c1 - c0] = cm[c0:c1]

        def dev(a):  # [CAP] -> [P, NT] with sample b at (b % P, b // P)
            return np.ascontiguousarray(a.reshape(NT, P).T)

        return {
            "xh": xh,
            "xt": dev(xtf),
            "keyt": dev(ktf),
            "wv": dev(wvf),
            "cmbase": dev(cbf),
            "cmr": cmr,
        }

    return [shard(c) for c in range(N_CORES)]


_NC_CACHE = None


def kernel(outputs: np.ndarray, targets: np.ndarray, cost_matrix: np.ndarray) -> np.ndarray:
    global _NC_CACHE
    assert outputs.shape == (B, C) and cost_matrix.shape == (C, C)
    if _NC_CACHE is None:
        _NC_CACHE = build_kernel()
    nc = _NC_CACHE
    in_maps = make_inputs(outputs, targets, cost_matrix)
    res = run_bass_kernel_spmd(nc, in_maps, core_ids=list(range(N_CORES)))
    total = sum(float(res.results[c]["partial"][0, 0]) for c in range(N_CORES))
    loss = -BETA * total / B
    return np.asarray(loss, dtype=np.float32)
